# revision 22
# baseline (speedup 1.0000x reference)
"""DeepSeekMoE (7 routed experts top-3 + 1 shared expert) on 8 NeuronCores.

Strategy: expert-parallel with host-side dispatch.
  - Host computes the (cheap) routing: softmax gate over 7 experts, top-3.
  - Cores 0-6 each own one routed expert's weights (slot A, capacity 3544
    tokens; ~3500 tokens route to each expert) plus a 581-token chunk of the
    shared expert (slot B). Core 7 runs the shared expert on its remaining
    4125 tokens. 8192 shared tokens = 7*581 + 4125.
  - Device kernel (same NEFF on all 8 cores) computes
        Y = (silu(X @ W1) * (X @ Wg)) @ W2 * scale
    for its 4125 gathered tokens in bf16 (f32 PSUM accumulation), where
    `scale` is the per-token combine weight (normalized top-k gate proba for
    routed slots, 1.0 for shared slots).
  - Host scatter-adds the per-core outputs into the full [B,S,D] result.

Scheduling notes (v3):
  - y is stored bf16 (halves the output DMA, which bounds the phase-2
    tail); host combine accumulates in f32. ht stores are paired (two
    h-tiles per dma_start) and the consumer pools are one buffer deeper
    to keep store completion off the PSUM-WAR critical path.
  - Phase-1 weights stream on the sync HWDGE queue in 256-column chunks,
    w1/wg interleaved, so the first matmuls start ~8us after kernel start
    instead of ~56us (waiting for the full 16MB load). x blocks ride the
    GpSimd SWDGE queue in parallel, prefetched one block ahead.
  - The first d-half of slot A's W2 prefetches on the GpSimd queue during
    phase-1 slot B, removing the phase-boundary stall. Phase-2 d-subtiles
    0/1 only touch that half; the second half and slot B's W2 stream in
    during early phase-2 compute.
"""

import threading

import numpy as np
import ml_dtypes

import concourse.bacc as bacc
import concourse.mybir as mybir
import concourse.tile as tile
from concourse.bass_utils import run_bass_kernel_spmd

BF16 = mybir.dt.bfloat16
F32 = mybir.dt.float32
NP_BF16 = ml_dtypes.bfloat16

B, S, D, H = 4, 2048, 2048, 2048
E, TOPK = 7, 3
NTOK = B * S                  # 8192 tokens
T_A, T_B = 3544, 581          # per-core slot capacities (see below)
T = T_A + T_B                 # 4125 tokens per core
KT = D // 128                 # 16 contraction k-tiles for GEMM1
HKT = H // 128                # 16 contraction k-tiles for GEMM2
N_CORES = 8
# token blocks: (column offset, width, slot).
# T_A >= max tokens per routed expert (3542 for the benchmark inputs);
# T_A + 8*T_B == NTOK so the shared expert is covered exactly:
# core 7 runs shared on all its 4125 tokens, cores 0-6 on 581 each.
BLOCKS = ([(i * 512, 512, 0) for i in range(6)] + [(3072, 472, 0)]
          + [(3544, 512, 1), (4056, 69, 1)])
# scale table is block-padded (512 slots per block) so each token subtile's
# scales live in an aligned 128-column regardless of ragged block offsets
SC_COLS = len(BLOCKS) * 4

TRACE = False                 # set by test harness to capture a profile
LAST_RESULT = None            # BassKernelResults of the last run

_nc_cache = {}
_nc_lock = threading.Lock()


def _build_nc(loop_k=1, loop_k2=None, loop_all=None):
    """Build + schedule the per-core Bass module (one NEFF, SPMD on 8 cores).

    loop_k > 1 wraps phase 1 (and loop_k2 phase 2; defaults to loop_k) in a
    hardware For_i loop that repeats the (idempotent) body — timing only.
    loop_all > 1 instead wraps ONE For_i around both phases (per-phase loops
    disabled); this is the trustworthy timing config.
    """
    if loop_k2 is None:
        loop_k2 = loop_k
    if loop_all is not None and loop_all > 1:
        loop_k = loop_k2 = 1
    import contextlib

    nc = bacc.Bacc("TRN2", target_bir_lowering=False, debug=False,
                   num_devices=N_CORES)

    # GEMM1 weights arrive host-swizzled as [128p, 8hc, 16k, 256h] so one
    # h-chunk is a single contiguous-per-partition (8KB lines) 1MB DMA.
    # x arrives as [128p, 9blk, 16k, 512t] (ragged blocks zero-padded): one
    # contiguous 2MB DMA per token block.
    xt = nc.dram_tensor("xt", [128, len(BLOCKS), KT, 512], BF16,
                        kind="ExternalInput")
    w1a = nc.dram_tensor("w1a", [128, H // 256, KT, 256], BF16,
                         kind="ExternalInput")
    wga = nc.dram_tensor("wga", [128, H // 256, KT, 256], BF16,
                         kind="ExternalInput")
    w2a = nc.dram_tensor("w2a", [128, D // 512, HKT, 512], BF16,
                         kind="ExternalInput")
    w1b = nc.dram_tensor("w1b", [128, H // 256, KT, 256], BF16,
                         kind="ExternalInput")
    wgb = nc.dram_tensor("wgb", [128, H // 256, KT, 256], BF16,
                         kind="ExternalInput")
    w2b = nc.dram_tensor("w2b", [128, D // 512, HKT, 512], BF16,
                         kind="ExternalInput")
    sc = nc.dram_tensor("sc", [128, SC_COLS], F32, kind="ExternalInput")
    # y in bf16: halves the 33.8MB output-store DMA, which bounds the
    # phase-2 tail; host accumulates in f32 (adds ~0.4% rounding, budget 2e-2)
    y = nc.dram_tensor("y", [T, D], BF16, kind="ExternalOutput")

    slots_1 = [(w1a, wga, 0), (w1b, wgb, 1)]
    blocks_p1 = [b for b in BLOCKS if b[2] == 0] + \
                [b for b in BLOCKS if b[2] == 1]

    def load_w1_wg(w1_sb, wg_sb, w1_d, wg_d, after_first=None):
        """Stream both GEMM1 weight matrices, interleaved per 256-col chunk
        (one 1MB DMA each) so early h-tiles become computable quickly. The
        first chunk is split into 128-col halves to start compute sooner."""
        for hc in range(H // 256):
            if hc == 0:
                for mat_sb, mat_d in ((w1_sb, w1_d), (wg_sb, wg_d)):
                    nc.sync.dma_start(mat_sb[:, 0, :, :128],
                                      mat_d[:, 0, :, :128])
                for mat_sb, mat_d in ((w1_sb, w1_d), (wg_sb, wg_d)):
                    nc.sync.dma_start(mat_sb[:, 0, :, 128:],
                                      mat_d[:, 0, :, 128:])
            else:
                nc.sync.dma_start(w1_sb[:, hc], w1_d[:, hc])
                nc.sync.dma_start(wg_sb[:, hc], wg_d[:, hc])
            if hc == 0 and after_first is not None:
                after_first()

    with tile.TileContext(nc) as tc:
        with tc.tile_pool(name="dram", bufs=1, space="DRAM") as dpool:
            ht_dram = dpool.tile([H, T], BF16)

            with tc.tile_pool(name="w2e", bufs=1) as w2epool, \
                 tc.tile_pool(name="hte", bufs=1) as htepool:
                # W2 arrives host-swizzled as [128p, 4dq, 16k, 512d]; the
                # first d-quarter of slot A prefetches during phase 1.
                w2a_lo = w2epool.tile([128, 1, HKT, 512], BF16)
                # block 0's HT readback gets dedicated SBUF so its DMA
                # doesn't WAR-wait on phase-1 pool space at the phase boundary
                ht_pre0 = htepool.tile([128, HKT, 512], BF16)
                # per-token scales are tiny and dependency-free: load once at
                # kernel start so phase 2's scale-mults never wait on them
                sc_sb = w2epool.tile([128, SC_COLS], F32, tag="sc")
                nc.gpsimd.dma_start(sc_sb[:], sc[:, :])

                outer_loop = (tc.For_i(0, loop_all, 1)
                              if loop_all is not None and loop_all > 1
                              else contextlib.nullcontext())
                outer_loop.__enter__()

                # ---- Phase 1: HT[h,t] = silu(x@W1).T * (x@Wg).T (bf16) ----
                with tc.tile_pool(name="w1p", bufs=1) as wpool, \
                     tc.tile_pool(name="xp", bufs=2) as xpool, \
                     tc.tile_pool(name="hp", bufs=4) as hpool, \
                     tc.tile_pool(name="hs", bufs=2) as hspool, \
                     tc.tile_pool(name="ps1", bufs=4, space="PSUM") as pspool, \
                     (tc.For_i(0, loop_k, 1) if loop_k > 1
                      else contextlib.nullcontext()):
                    xt_tiles = {}

                    def fetch_x(bi):
                        if bi in xt_tiles or bi >= len(blocks_p1):
                            return
                        t_ = xpool.tile([128, KT, 512], BF16, tag="xt")
                        nc.gpsimd.dma_start(t_[:], xt[:, bi])
                        xt_tiles[bi] = t_

                    bi = 0
                    for w1_d, wg_d, slot in slots_1:
                        w1_sb = wpool.tile([128, H // 256, KT, 256], BF16,
                                           tag="w1")
                        wg_sb = wpool.tile([128, H // 256, KT, 256], BF16,
                                           tag="wg")
                        if slot == 0:
                            fetch_x(0)
                            load_w1_wg(w1_sb, wg_sb, w1_d, wg_d)
                        else:
                            # phase-2 prefetches during slot B: the first two
                            # d-quarters of slot A's W2 ride the GpSimd
                            # queue; block 0's HT readback goes on sync right
                            # after the first slot-B weight chunk pair
                            nc.gpsimd.dma_start(w2a_lo[:], w2a[:, 0:1])

                            def _pre():
                                c0, bw, _ = blocks_p1[0]
                                ht_r = ht_dram[:, c0:c0 + bw].rearrange(
                                    "(ko p) t -> p ko t", p=128)
                                nc.sync.dma_start(ht_pre0[:, :, :bw], ht_r)

                            load_w1_wg(w1_sb, wg_sb, w1_d, wg_d,
                                       after_first=_pre)
                        for c0, bw, bslot in blocks_p1:
                            if bslot != slot:
                                continue
                            fetch_x(bi)
                            fetch_x(bi + 1)
                            xt_sb = xt_tiles.pop(bi)
                            bi += 1
                            hpt2 = None
                            for h in range(H // 128):
                                hc, sub = h // 2, (h % 2) * 128
                                hs = slice(sub, sub + 128)
                                ps_1 = pspool.tile([128, 512], F32, tag="ps1")
                                for k in range(KT):
                                    nc.tensor.matmul(
                                        ps_1[:, :bw], w1_sb[:, hc, k, hs],
                                        xt_sb[:, k, :bw],
                                        start=(k == 0), stop=(k == KT - 1))
                                ps_g = pspool.tile([128, 512], F32, tag="psg")
                                for k in range(KT):
                                    nc.tensor.matmul(
                                        ps_g[:, :bw], wg_sb[:, hc, k, hs],
                                        xt_sb[:, k, :bw],
                                        start=(k == 0), stop=(k == KT - 1))
                                sil = hpool.tile([128, 512], BF16, tag="sil")
                                nc.scalar.activation(
                                    sil[:, :bw], ps_1[:, :bw],
                                    mybir.ActivationFunctionType.Silu)
                                if h % 2 == 0:
                                    hpt2 = hspool.tile([128, 2, 512], BF16,
                                                       tag="ht")
                                nc.vector.tensor_tensor(
                                    hpt2[:, h % 2, :bw], sil[:, :bw],
                                    ps_g[:, :bw], mybir.AluOpType.mult)
                                if h % 2 == 1:
                                    # paired store (two h-tiles per dma_start)
                                    # on the scalar HWDGE queue so it doesn't
                                    # contend with the sync-queue loads
                                    ht_w = ht_dram[
                                        (h - 1) * 128:(h + 1) * 128,
                                        c0:c0 + bw].rearrange(
                                        "(ko p) t -> p ko t", p=128)
                                    nc.scalar.dma_start(
                                        ht_w, hpt2[:, :, :bw])

                # ---- Phase 2: Y[t,d] = (HT.T @ W2) * scale[t] (f32 out) ----
                with tc.tile_pool(name="w2p", bufs=1) as w2pool, \
                     tc.tile_pool(name="hp2", bufs=3) as hpool2, \
                     tc.tile_pool(name="yp", bufs=6) as ypool, \
                     tc.tile_pool(name="ps2", bufs=8, space="PSUM") as pspool2, \
                     (tc.For_i(0, loop_k, 1) if loop_k > 1
                      else contextlib.nullcontext()):
                    ht_tiles = {0: ht_pre0}

                    def fetch_ht(bi):
                        if bi in ht_tiles or bi >= len(blocks_p1):
                            return
                        c0, bw, _ = blocks_p1[bi]
                        t_ = hpool2.tile([128, HKT, 512], BF16, tag="ht2")
                        ht_r = ht_dram[:, c0:c0 + bw].rearrange(
                            "(ko p) t -> p ko t", p=128)
                        nc.sync.dma_start(t_[:, :, :bw], ht_r)
                        ht_tiles[bi] = t_

                    fetch_ht(1)
                    # remaining d-quarters of slot A's W2 on sync, slot B's
                    # full W2 on the GpSimd queue
                    w2a_hi = w2pool.tile([128, 3, HKT, 512], BF16,
                                         tag="w2ahi")
                    for q in range(3):
                        nc.sync.dma_start(w2a_hi[:, q], w2a[:, q + 1])
                    w2b_sb = w2pool.tile([128, 4, HKT, 512], BF16, tag="w2b")
                    nc.gpsimd.dma_start(w2b_sb[:], w2b[:])

                    def w2_src(slot, j):
                        """SBUF source for output d-subtile j (512 wide)."""
                        if slot == 1:
                            return w2b_sb[:, j]
                        if j == 0:
                            return w2a_lo[:, 0]
                        return w2a_hi[:, j - 1]

                    for bi, (c0, bw, slot) in enumerate(blocks_p1):
                        fetch_ht(bi)
                        fetch_ht(bi + 1)
                        ht_sb = ht_tiles.pop(bi)
                        # d-subtile-outer so only quarter 0 is needed during
                        # the first ~14us while quarters 1-3 stream in
                        for j in range(D // 512):           # output d subtiles
                            w2_t = w2_src(slot, j)
                            for i in range((bw + 127) // 128):  # token subtile
                                tw = min(128, bw - i * 128)
                                ts_ = slice(i * 128, i * 128 + tw)
                                psy = pspool2.tile([128, 512], F32, tag="psy")
                                for k in range(HKT):
                                    nc.tensor.matmul(
                                        psy[:tw], ht_sb[:, k, ts_],
                                        w2_t[:, k, :],
                                        start=(k == 0), stop=(k == HKT - 1))
                                yt_sb = ypool.tile([128, 512], BF16, tag="y")
                                col = bi * 4 + i
                                nc.vector.tensor_scalar_mul(
                                    yt_sb[:tw], psy[:tw],
                                    sc_sb[:tw, col:col + 1])
                                nc.scalar.dma_start(
                                    y[c0 + i * 128:c0 + i * 128 + tw,
                                      j * 512:(j + 1) * 512],
                                    yt_sb[:tw])
                outer_loop.__exit__(None, None, None)
    nc.compile()
    return nc


def _get_nc(loop_k=1, loop_k2=None, loop_all=None):
    with _nc_lock:
        key = (loop_k, loop_k2, loop_all)
        if key not in _nc_cache:
            _nc_cache[key] = _build_nc(loop_k, loop_k2, loop_all)
        return _nc_cache[key]


def benchmark(in_maps, iters=8, loop_k=1, loop_k2=None, loop_all=None):
    """Time the NEFF execution with device-resident inputs.

    Returns (best_wall_seconds_per_call, outputs_list). With loop_k > 1 the
    NEFF repeats the kernel body loop_k times on-device; comparing against
    loop_k=1 cancels the (large, ~100ms) axon dispatch overhead.
    """
    import time as _time

    import jax
    from jax.sharding import Mesh, NamedSharding, PartitionSpec
    from jax.experimental.shard_map import shard_map

    from concourse import bass2jax, mybir as _mybir

    nc = _get_nc(loop_k, loop_k2, loop_all)
    bass2jax.install_neuronx_cc_hook()

    partition_name = (nc.partition_id_tensor.name
                      if nc.partition_id_tensor else None)
    in_names, out_names, out_avals, zero_outs = [], [], [], []
    for alloc in nc.m.functions[0].allocations:
        if not isinstance(alloc, _mybir.MemoryLocationSet):
            continue
        name = alloc.memorylocations[0].name
        if alloc.kind == "ExternalInput":
            if name != partition_name:
                in_names.append(name)
        elif alloc.kind == "ExternalOutput":
            out_names.append(name)
            shape = tuple(alloc.tensor_shape)
            dtype = _mybir.dt.np(alloc.dtype)
            out_avals.append(jax.core.ShapedArray(shape, dtype))
            zero_outs.append(np.zeros(shape, dtype))
    n_params = len(in_names)
    all_names = in_names + out_names
    if partition_name is not None:
        all_names = all_names + [partition_name]

    def _exec_once(args, outs):
        extra = ([bass2jax.partition_id_tensor()]
                 if partition_name is not None else [])
        return bass2jax._bass_exec_p.bind(
            *args, *outs, *extra,
            out_avals=tuple(out_avals),
            in_names=tuple(all_names),
            out_names=tuple(out_names),
            lowering_input_output_aliases=(),
            sim_require_finite=True,
            sim_require_nnan=True,
            nc=nc,
        )

    def _body(*args):
        ins, outs = args[:n_params], list(args[n_params:])
        return tuple(_exec_once(ins, outs))

    n_cores = len(in_maps)
    devices = jax.devices()[:n_cores]
    mesh = Mesh(np.asarray(devices), ("core",))
    spec = PartitionSpec("core")
    sharded = jax.jit(
        shard_map(_body, mesh=mesh,
                  in_specs=(spec,) * (n_params + len(out_names)),
                  out_specs=(spec,) * len(out_names), check_rep=False),
        keep_unused=True)

    sh = NamedSharding(mesh, spec)
    dev_in = [
        jax.device_put(
            np.concatenate([np.asarray(in_maps[c][nm]) for c in range(n_cores)],
                           axis=0), sh)
        for nm in in_names
    ]
    dev_zero = [
        jax.device_put(np.zeros((n_cores * z.shape[0], *z.shape[1:]), z.dtype),
                       sh)
        for z in zero_outs
    ]
    out = sharded(*dev_in, *dev_zero)
    jax.block_until_ready(out)

    all_times = []
    for _ in range(iters):
        t0 = _time.perf_counter()
        out = sharded(*dev_in, *dev_zero)
        jax.block_until_ready(out)
        all_times.append(_time.perf_counter() - t0)
    best = min(all_times)
    benchmark.last_times = all_times

    results = [
        {nm: np.asarray(out[i]).reshape(n_cores, *out_avals[i].shape)[c]
         for i, nm in enumerate(out_names)}
        for c in range(n_cores)
    ]
    return best, results


def _softmax_f32(x):
    m = x.max(axis=-1, keepdims=True)
    e = np.exp((x - m).astype(np.float64))
    return (e / e.sum(axis=-1, keepdims=True)).astype(np.float32)


def _np_ffn(x, w1, wg, w2):
    h1 = x @ w1
    return ((h1 / (1.0 + np.exp(-h1))) * (x @ wg)) @ w2


def _dispatch(x, W1, Wg, W2, Ws1, Wsg, Ws2, gate_w, gate_b, biases):
    """Host-side routing + sharding. Returns (in_maps, core_idx, overflow, xf)."""
    x = np.asarray(x, dtype=np.float32)
    W1 = np.asarray(W1, dtype=np.float32)
    Wg = np.asarray(Wg, dtype=np.float32)
    W2 = np.asarray(W2, dtype=np.float32)
    Ws1 = np.asarray(Ws1, dtype=np.float32)
    Wsg = np.asarray(Wsg, dtype=np.float32)
    Ws2 = np.asarray(Ws2, dtype=np.float32)
    gate_w = np.asarray(gate_w, dtype=np.float32)
    gate_b = np.asarray(gate_b, dtype=np.float32)
    biases = np.asarray(biases, dtype=np.float32)

    xf = x.reshape(NTOK, D)

    # ---- routing (host): mirrors the reference math ----
    logits = xf @ gate_w + gate_b
    probas = _softmax_f32(logits)
    biased = probas + biases
    # jax.lax.top_k tie-break: lowest index first -> stable argsort of -biased
    topk = np.argsort(-biased, axis=-1, kind="stable")[:, :TOPK]
    tp = np.take_along_axis(probas, topk, axis=-1)
    tp = tp / tp.sum(axis=-1, keepdims=True)

    # ---- dispatch ----
    def _swz1(w):
        # [D, H] -> [128p, 8hc, 16k, 256h]: d = k*128 + p, h = hc*256 + hh
        return np.ascontiguousarray(
            w.astype(NP_BF16).reshape(KT, 128, H // 256, 256)
            .transpose(1, 2, 0, 3))

    def _swz2(w):
        # [H, D] -> [128p, 4dq, 16k, 512d]: h = k*128 + p, d = dq*512 + dd
        return np.ascontiguousarray(
            w.astype(NP_BF16).reshape(HKT, 128, D // 512, 512)
            .transpose(1, 2, 0, 3))

    xbf = xf.astype(NP_BF16)
    w1bf = [_swz1(W1[e]) for e in range(E)]
    wgbf = [_swz1(Wg[e]) for e in range(E)]
    w2bf = [_swz2(W2[e]) for e in range(E)]
    ws1bf, wsgbf, ws2bf = (_swz1(Ws1), _swz1(Wsg), _swz2(Ws2))

    expert_tok = []   # token ids routed to expert e
    expert_wt = []    # their combine weights
    overflow = []     # (token, expert, weight) pairs beyond slot capacity
    for e in range(E):
        sel = (topk == e)
        rows = np.where(sel.any(axis=-1))[0]
        wts = (tp * sel).sum(axis=-1)[rows]
        if len(rows) > T_A:
            for t, w in zip(rows[T_A:], wts[T_A:]):
                overflow.append((int(t), e, float(w)))
            rows, wts = rows[:T_A], wts[:T_A]
        expert_tok.append(rows)
        expert_wt.append(wts.astype(np.float32))

    shared_chunks = [np.arange(T + T_B * i, T + T_B * (i + 1))
                     for i in range(E)]          # cores 0-6: T_B tokens each
    shared_chunks.append(np.arange(0, T))        # core 7: T tokens

    in_maps = []
    core_idx = []   # (idxA, nA, idxB, nB) for the combine step
    for c in range(N_CORES):
        if c < E:
            idx_a, wt_a = expert_tok[c], expert_wt[c]
            w1s, wgs, w2s = w1bf[c], wgbf[c], w2bf[c]
            idx_b = shared_chunks[c]
        else:
            idx_a = shared_chunks[c][:T_A]
            wt_a = np.ones(T_A, np.float32)
            w1s, wgs, w2s = ws1bf, wsgbf, ws2bf
            idx_b = shared_chunks[c][T_A:]
        n_a, n_b = len(idx_a), len(idx_b)

        xg = np.zeros((len(BLOCKS) * 512, D), dtype=NP_BF16)
        gathered = np.zeros((T, D), dtype=NP_BF16)
        gathered[:n_a] = xbf[idx_a]
        gathered[T_A:T_A + n_b] = xbf[idx_b]
        for bi, (c0, bw, _) in enumerate(BLOCKS):
            xg[bi * 512:bi * 512 + bw] = gathered[c0:c0 + bw]
        # [9*512, D] -> [128p, 9blk, 16k, 512t]
        xt_c = np.ascontiguousarray(
            xg.reshape(len(BLOCKS), 512, KT, 128).transpose(3, 0, 2, 1))

        s = np.zeros(T, np.float32)
        s[:n_a] = wt_a
        s[T_A:T_A + n_b] = 1.0
        # block-padded: 512 slots (4 columns of 128) per block
        s_pad = np.zeros(SC_COLS * 128, np.float32)
        for bi_, (c0_, bw_, _) in enumerate(BLOCKS):
            s_pad[bi_ * 512:bi_ * 512 + bw_] = s[c0_:c0_ + bw_]
        sc_c = np.ascontiguousarray(s_pad.reshape(SC_COLS, 128).T)

        in_maps.append({
            "xt": xt_c, "sc": sc_c,
            "w1a": w1s, "wga": wgs, "w2a": w2s,
            "w1b": ws1bf, "wgb": wsgbf, "w2b": ws2bf,
        })
        core_idx.append((idx_a, n_a, idx_b, n_b))

    return in_maps, core_idx, overflow, xf


def _combine(results, core_idx, overflow, xf, W1, Wg, W2):
    out = np.zeros((NTOK, D), np.float32)
    for c in range(N_CORES):
        yc = np.asarray(results[c]["y"], dtype=np.float32)
        idx_a, n_a, idx_b, n_b = core_idx[c]
        out[idx_a] += yc[:n_a]
        out[idx_b] += yc[T_A:T_A + n_b]

    # correctness fallback if an expert exceeded slot capacity (never happens
    # for the benchmark distribution, but keeps the kernel total-correct)
    for t, e, w in overflow:
        out[t] += w * _np_ffn(xf[t:t + 1], np.asarray(W1[e], np.float32),
                              np.asarray(Wg[e], np.float32),
                              np.asarray(W2[e], np.float32))[0]

    return out.reshape(B, S, D)


def kernel(x, W1, Wg, W2, Ws1, Wsg, Ws2, gate_w, gate_b, biases):
    global LAST_RESULT
    in_maps, core_idx, overflow, xf = _dispatch(
        x, W1, Wg, W2, Ws1, Wsg, Ws2, gate_w, gate_b, biases)

    nc = _get_nc()
    res = run_bass_kernel_spmd(nc, in_maps, core_ids=list(range(N_CORES)))
    LAST_RESULT = res

    return _combine(res.results, core_idx, overflow, xf, W1, Wg, W2)

